# revision 7
# baseline (speedup 1.0000x reference)
"""Trainium2 kernel for the OpticalFront dense net.

Reference computation:
    xr = Re(idft2(tmask * dft2(x)))          # centered 2D FFT front
    h = relu(xr.flat @ w1.T + b1)
    out = log_softmax(h @ w4.T + b4)

The optical front is a fixed real-linear operator A on each flattened
28x28 image (xr_flat = x_flat @ A.T), so it folds into the first FC
layer on the host: w1_eff = w1 @ A.  The device then runs a pure GEMM
pipeline, data-parallel over 8 NeuronCores (4096 images per core).

FC1 runs entirely in fp8-e4m3 DoubleRow mode (2 contraction rows per
PE cell, ~2x the bf16 matmul rate): pixels 0..767 in 3 DoubleRow
matmuls of 256 virtual rows, pixels 768..783 plus the bias row in one
thin 9-partition DoubleRow matmul, all accumulating into one PSUM
bank.  Keeping every FC1 matmul in the same fp8 mode avoids the PE
drain/refill that mixed-dtype back-to-back matmuls cost.  w1 is
pre-scaled by 64 so its fp8 encoding sits in e4m3's normal range; the
1/64 is folded into w4 on the host.  FC2 stays bf16 (fp8 there would
blow the error budget), and the log-softmax runs entirely off the PE:
GpSimd reduces exp() across the 10 class partitions, ScalarE takes the
log, and DVE applies bias and subtraction.

    H'.T[hid, b] = sum_t W18[t].T @ X8[t]                   (fp32 acc)
    L.T[10,  b] = sum_k W4T[k, 10].T  @ H'.T[k, b]          (w4/64)
    out.T[10, b] = (L.T + b4) - ln(allreduce_p(exp(L.T + b4)))

Layout: contraction dims on the SBUF partition axis; batch streams
along the free axis in chunks of 512 (one PSUM bank), each chunk's
x-data contiguous per partition in DRAM (3KB DMA lines).
"""

import numpy as np
import ml_dtypes

import concourse.bass as bass
import concourse.bass_isa as bass_isa
import concourse.bacc as bacc
import concourse.mybir as mybir
import concourse.tile as tile
from concourse.bass_utils import run_bass_kernel_spmd

BF16 = mybir.dt.bfloat16
FP8 = mybir.dt.float8e4
F32 = mybir.dt.float32
AF = mybir.ActivationFunctionType
DR = mybir.MatmulPerfMode.DoubleRow

B, H, W = 32768, 28, 28
PIX = H * W            # 784
HID = 800
NCLS = 10
NCORES = 8
BPC = B // NCORES      # 4096 images per core
NB = 512               # batch chunk = one PSUM bank of fp32
NCH = BPC // NB        # 8 chunks per core
NT8 = 3                # full fp8 DoubleRow tiles (256 pixels each = 768)
P8 = NT8 * 256         # pixels covered by the full fp8 tiles
KT4 = 9                # thin tile partitions: 18 virtual rows >= 16 px + bias
WS = 64.0              # host-side scale on w1/b1 (folded out via w4)
HT = (HID + 127) // 128          # 7 contraction tiles for fc2
M_TILES = [(m * 128, min(128, HID - m * 128)) for m in range(HT)]

_built = None  # nc cache — BIR build is pure host work


def _build_device_program():
    nc = bacc.Bacc(
        "TRN2", target_bir_lowering=False, debug=False, num_devices=NCORES
    )
    # x8 packed per compute chunk: per partition p, chunk nb holds the
    # contiguous 3KB block [t, i, n] with pixel k = 256 t + 2 p + i.
    x8_d = nc.dram_tensor("x8", [128, NCH, NT8, 2, NB], FP8, kind="ExternalInput")
    # thin tail tile: virtual row v = 2 p + i -> pixel 768+v (v<16),
    # v==16 the ones/bias row, v==17 zero pad.  Loaded whole at startup.
    x4_d = nc.dram_tensor("x4", [KT4, 2, BPC], FP8, kind="ExternalInput")
    w18_d = nc.dram_tensor("w18", [128, NT8, 2, HID], FP8, kind="ExternalInput")
    w4_8d = nc.dram_tensor("wt4", [KT4, 2, HID], FP8, kind="ExternalInput")
    w4t_d = nc.dram_tensor("w4t", [HT * 128, NCLS], BF16, kind="ExternalInput")
    b4_d = nc.dram_tensor("b4", [NCLS, 1], F32, kind="ExternalInput")
    out_d = nc.dram_tensor("outT", [NCLS, BPC], F32, kind="ExternalOutput")

    # The one ACT-function table containing relu/exp/ln/identity (avoids
    # per-transition LUT reloads).  Loaded AFTER the first DMAs are issued
    # on the scalar queue so it doesn't delay the first matmul's weights.
    from concourse.hw_specs import get_activation_tables
    needed = {AF.Relu, AF.Exp, AF.Ln, AF.Identity, AF.Copy}
    table_id = None
    for i, (name, funcs) in enumerate(get_activation_tables(nc.m.arch).items()):
        if needed <= funcs:
            table_id = i
            break

    with tile.TileContext(nc) as tc:
        with (
            tc.tile_pool(name="weights", bufs=1) as wpool,
            tc.tile_pool(name="xin", bufs=3) as xpool,
            tc.tile_pool(name="hmid", bufs=2 * HT) as hpool,
            tc.tile_pool(name="smax", bufs=8) as spool,
            tc.tile_pool(name="psum_h", bufs=3, space="PSUM") as psum_h,
            tc.tile_pool(name="psum_l", bufs=3, space="PSUM") as psum_l_pool,
        ):
            w4_view = w4t_d.ap().rearrange("(k p) m -> p k m", p=128)

            # Startup order: the first m-group needs w18 t0 (scalar
            # queue) and chunk 0 of x8 (sync queue); issue those first
            # on their queues, everything else after.
            w18_sb = wpool.tile([128, NT8, 2, HID], FP8)
            nc.scalar.dma_start(w18_sb[:, 0:1, :, :], w18_d[:, 0:1, :, :])

            xts = {}   # compute chunk -> [128, NT8, 2, NB] tile

            def load_chunk(nb):
                x8_sb = xpool.tile([128, NT8, 2, NB], FP8, tag="x8")
                nc.sync.dma_start(x8_sb[:, :, :, :], x8_d[:, nb, :, :, :])
                xts[nb] = x8_sb

            load_chunk(0)

            nc.scalar.dma_start(w18_sb[:, 1:NT8, :, :], w18_d[:, 1:NT8, :, :])
            wt4_sb = wpool.tile([KT4, 2, HID], FP8)
            nc.scalar.dma_start(wt4_sb[:, :, :], w4_8d[:, :, :])

            if table_id is not None:
                nc.scalar.add_instruction(
                    mybir.InstLoadActFuncSet(
                        name=nc.get_next_instruction_name(),
                        act_func_set_id=table_id,
                        ins=[],
                        outs=[],
                    )
                )

            load_chunk(1)
            x4_sb = wpool.tile([KT4, 2, BPC], FP8)
            nc.sync.dma_start(x4_sb[:, :, :], x4_d[:, :, :])

            w4_sb = wpool.tile([128, HT, NCLS], BF16)
            nc.gpsimd.dma_start(w4_sb[:, :, :], w4_view)
            b4_sb = wpool.tile([NCLS, 1], F32)
            nc.gpsimd.dma_start(b4_sb[:, :], b4_d[:, :])

            for nb in range(NCH):
                if nb + 2 < NCH:
                    load_chunk(nb + 2)
                gs = slice(nb * NB, (nb + 1) * NB)

                hts = []
                for m, (m0, mm) in enumerate(M_TILES):
                    ph = psum_h.tile([128, NB], F32, tag="ph")
                    for t in range(NT8):
                        nc.tensor.matmul(
                            ph[:mm, :],
                            w18_sb[:, t, :, m0:m0 + mm],
                            xts[nb][:, t, :, :],
                            start=(t == 0),
                            stop=False,
                            perf_mode=DR,
                        )
                    nc.tensor.matmul(
                        ph[:mm, :],
                        wt4_sb[:, :, m0:m0 + mm],
                        x4_sb[:, :, gs],
                        start=False,
                        stop=True,
                        perf_mode=DR,
                    )
                    ht = hpool.tile([128, NB], BF16, tag="ht")
                    nc.scalar.activation(ht[:mm, :], ph[:mm, :], AF.Relu)
                    hts.append(ht)

                pl = psum_l_pool.tile([NCLS, NB], F32, tag="pl")
                for k in range(HT):
                    kk = min(128, HID - k * 128)
                    nc.tensor.matmul(
                        pl[:, :],
                        w4_sb[:kk, k, :],
                        hts[k][:kk, :],
                        start=(k == 0),
                        stop=(k == HT - 1),
                    )

                # log-softmax entirely off the PE: exp on ScalarE, class
                # sum via GpSimd partition all-reduce, ln on ScalarE,
                # bias-add and subtract on DVE.
                exp_sb = spool.tile([NCLS, NB], BF16, tag="exp")
                nc.scalar.activation(exp_sb[:, :], pl[:, :], AF.Exp, bias=b4_sb[:, :])
                sexp_sb = spool.tile([NCLS, NB], F32, tag="sexp")
                nc.gpsimd.partition_all_reduce(
                    sexp_sb[:, :], exp_sb[:, :], channels=NCLS,
                    reduce_op=bass_isa.ReduceOp.add,
                )
                lse_sb = spool.tile([NCLS, NB], BF16, tag="lse")
                nc.scalar.activation(lse_sb[:, :], sexp_sb[:, :], AF.Ln)
                logit_sb = spool.tile([NCLS, NB], F32, tag="logit")
                nc.vector.tensor_scalar_add(logit_sb[:, :], pl[:, :], b4_sb[:, :])
                out_sb = spool.tile([NCLS, NB], F32, tag="outc")
                nc.vector.tensor_sub(out_sb[:, :], logit_sb[:, :], lse_sb[:, :])
                nc.scalar.dma_start(out_d[:, gs], out_sb[:, :])

    nc.finalize()
    return nc


def _optical_operator(tmask_re, tmask_im):
    """A such that xr_flat = A @ x_flat for the masked centered FFT front."""
    tmask = tmask_re.astype(np.complex64) + 1j * tmask_im.astype(np.complex64)
    tmask = tmask.reshape(H, W)
    ax = (-2, -1)
    eye = np.eye(PIX, dtype=np.complex64).reshape(PIX, H, W)
    f = np.fft.fftshift(np.fft.fft2(np.fft.ifftshift(eye, axes=ax), axes=ax), axes=ax)
    f *= tmask[None, :, :]
    xr = np.fft.fftshift(np.fft.ifft2(np.fft.ifftshift(f, axes=ax), axes=ax), axes=ax)
    return np.real(xr).reshape(PIX, PIX).T.astype(np.float64)


def kernel(x, tmask_re, tmask_im, w1, b1, w4, b4):
    global _built
    x = np.asarray(x)
    w1 = np.asarray(w1, dtype=np.float32)
    b1 = np.asarray(b1, dtype=np.float32)
    w4 = np.asarray(w4, dtype=np.float32)
    b4 = np.asarray(b4, dtype=np.float32)
    tre = np.asarray(tmask_re, dtype=np.float32)
    tim = np.asarray(tmask_im, dtype=np.float32)

    # Fold the optical front into w1.  Identity mask -> A == I exactly.
    if np.all(tre == 1.0) and np.all(tim == 0.0):
        w1e = w1.astype(np.float64)
    else:
        w1e = w1.astype(np.float64) @ _optical_operator(tre, tim)

    bf16 = ml_dtypes.bfloat16
    fp8 = ml_dtypes.float8_e4m3fn

    def q8(a):
        return np.clip(a, -240, 240).astype(fp8)

    # w1 scaled by WS so fp8 encodings sit in e4m3's normal range; the
    # matching 1/WS rides on w4 (relu commutes with positive scaling).
    w1s = (w1e * WS).astype(np.float32)
    # full fp8 tiles: [hid, 768] -> [128 p, 3 t, 2 i, hid], k = 256t+2p+i
    w18 = np.ascontiguousarray(
        q8(w1s[:, :P8]).reshape(HID, NT8, 128, 2).transpose(2, 1, 3, 0)
    )
    # thin tile: rows v = 2p+i -> pixel 768+v (v<16), v=16 bias, v=17 pad
    wt4f = np.zeros((2 * KT4, HID), dtype=np.float32)
    wt4f[:PIX - P8, :] = w1s[:, P8:].T
    wt4f[PIX - P8, :] = (b1 * WS).astype(np.float32)
    wt4 = np.ascontiguousarray(q8(wt4f).reshape(KT4, 2, HID))

    w4t = np.zeros((HT * 128, NCLS), dtype=bf16)
    w4t[:HID, :] = (w4 / WS).T
    b4c = np.ascontiguousarray(b4.reshape(NCLS, 1))

    # x: [B, 784] fp8, packed to match the weight tiles; x8 chunked so
    # each compute chunk is contiguous per partition (3KB DMA lines).
    xf = x.reshape(B, PIX)
    x8 = q8(xf[:, :P8]).reshape(B, NT8, 128, 2).transpose(2, 0, 1, 3)
    x4f = np.zeros((2 * KT4, B), dtype=np.float32)
    x4f[:PIX - P8, :] = xf[:, P8:].T
    x4f[PIX - P8, :] = 1.0
    x4 = q8(x4f).reshape(KT4, 2, B)

    if _built is None:
        _built = _build_device_program()
    nc = _built

    in_maps = []
    for c in range(NCORES):
        sl = slice(c * BPC, (c + 1) * BPC)
        # [128, BPC, 3, 2] -> [128, NCH, NB, 3, 2] -> [128, NCH, 3, 2, NB]
        x8c = x8[:, sl].reshape(128, NCH, NB, NT8, 2).transpose(0, 1, 3, 4, 2)
        in_maps.append({
            "x8": np.ascontiguousarray(x8c),
            "x4": np.ascontiguousarray(x4[:, :, sl]),
            "w18": w18,
            "wt4": wt4,
            "w4t": w4t,
            "b4": b4c,
        })
    res = run_bass_kernel_spmd(nc, in_maps, core_ids=list(range(NCORES)))

    out = np.empty((B, NCLS), dtype=np.float32)
    for c in range(NCORES):
        out[c * BPC:(c + 1) * BPC, :] = res.results[c]["outT"].T
    return out
